# revision 27
# baseline (speedup 1.0000x reference)
"""AdaptiveBoxBlur2d on 8 TRN2 NeuronCores (Bass/Tile).

Math: the reference normalizes each (n,c) image, builds a SAT (2D cumsum) and
samples it bilinearly at 4 per-pixel corners (box +/- half-sizes), then
rescales.  Identity: a bilinear-interp difference of a cumsum equals
convolution with a trapezoid window W(u) = clamp01(B'-u+1) - clamp01(A'-u+1),
A' = clamp(c0 - (k+1)*s, 0, D-1), B' = clamp(c0 + (k-1)*s, 0, D-1),
s = (D-1)/(2D).  The 2D op is the per-pixel product window
sum_{u,v} Wy(u)Wx(v) xn[u,v], support |u-h|<=8, |v-w|<=8 (k in [1,16)).
This gives an exact 17x17 per-pixel-weighted window sum with ANALYTIC
weights -- no data-dependent gathers, which Trainium has no fast path for.

Engine split: ACT builds the relu half of the clamp weights; DVE forms the
17 weighted-tap products in bf16 (2x_1P mode, same-parity tap offsets paired
into 9 fused 4D-AP instructions, with a 1-element-shifted copy of each row
band so every tap slice stays 4B-aligned); the PE accumulates
the 17 product planes into PSUM via identity-matmuls (f32 adds, frees the
DVE of the add chain); ACT evacuates PSUM to bf16 so the row-weight multiply
and accumulate run in 2x mode; row shifts are SBUF->SBUF DMAs.  Edge bands
skip the clamped-side halo (those taps have zero weight), packing H=1024
into 9 bands (120 + 7*112 + 120) instead of 10.
~4.14 ms/core by the calibrated TRN2 cost model; measured rel err 4.3e-3
(gate 2e-2).

Sharding: data-parallel over the 16 (n,c) channel-images, 2 per core
(cores 0-3 -> image 0, cores 4-7 -> image 1).  No collectives.
"""

import sys
from contextlib import ExitStack

import numpy as np

sys.path.insert(0, "/opt/trn_rl_repo")

N, C, H, W = 2, 8, 1024, 1024
EPS = 1e-5
SC = (W - 1) / (2.0 * W)  # 0.49951171875 (same for H)
PADW = 8
WP = W + 2 * PADW
NCH = 2                   # channels per core

_COMPILED = None


def build_bass():
    import concourse.bass as bass
    import concourse.tile as tile
    from concourse import bacc, mybir

    fp32 = mybir.dt.float32
    bf16 = mybir.dt.bfloat16
    AX = mybir.AxisListType
    OP = mybir.AluOpType
    AF = mybir.ActivationFunctionType

    nc = bacc.Bacc("TRN2", target_bir_lowering=False, debug=False)

    x_d = nc.dram_tensor("x", [NCH, H, W], fp32, kind="ExternalInput")
    ks_d = nc.dram_tensor("kernel_sizes", [H, W, 2], fp32, kind="ExternalInput")
    out_d = nc.dram_tensor("out", [NCH, H, W], fp32, kind="ExternalOutput")

    with tile.TileContext(nc) as tc, ExitStack() as ctx:
        singles = ctx.enter_context(tc.tile_pool(name="singles", bufs=1))
        coords_p = ctx.enter_context(tc.tile_pool(name="coords", bufs=1))
        wx_p = ctx.enter_context(tc.tile_pool(name="wx", bufs=1))
        work_p = ctx.enter_context(tc.tile_pool(name="work", bufs=2))
        sh_p = ctx.enter_context(tc.tile_pool(name="sh", bufs=2))
        tmp_p = ctx.enter_context(tc.tile_pool(name="tmp", bufs=2))
        acc_p = ctx.enter_context(tc.tile_pool(name="acc", bufs=2))

        # ---- constants ----
        iota_i = singles.tile([128, 1], mybir.dt.int32)
        nc.gpsimd.iota(iota_i, pattern=[[0, 1]], base=0, channel_multiplier=1)
        iota_col = singles.tile([128, 1], fp32)
        nc.vector.tensor_copy(out=iota_col, in_=iota_i)
        wrow_i = singles.tile([128, W], mybir.dt.int32)
        nc.gpsimd.iota(wrow_i, pattern=[[1, W]], base=0, channel_multiplier=0)
        wrow = singles.tile([128, W], fp32)
        nc.vector.tensor_copy(out=wrow, in_=wrow_i)
        ones_col = singles.tile([128, 1], fp32)
        nc.vector.memset(ones_col, 1.0)
        idrow_i = singles.tile([128, 128], mybir.dt.int32)
        nc.gpsimd.iota(idrow_i, pattern=[[1, 128]], base=0, channel_multiplier=0)
        idrow_f = singles.tile([128, 128], fp32)
        nc.vector.tensor_copy(out=idrow_f, in_=idrow_i)
        ident = singles.tile([128, 128], bf16)
        nc.vector.tensor_scalar(out=ident, in0=idrow_f, scalar1=iota_col,
                                scalar2=None, op0=OP.is_equal)
        shift_bias = singles.tile([128, 17], fp32)
        for j in range(17):
            nc.vector.memset(shift_bias[:, j:j + 1], float(9 - j))
        # per-channel scalars on partition 0: [s1, nb, s2, m] x NCH
        scal = singles.tile([1, NCH * 4], fp32)
        bcast = singles.tile([128, NCH * 4], fp32)

        # ---- pass 1: per-channel mean/std over the full image ----
        p1 = ExitStack()
        stats_p = p1.enter_context(tc.tile_pool(name="stats", bufs=2))
        psum_p = p1.enter_context(tc.tile_pool(name="ps", bufs=2, space="PSUM"))
        xload_p = p1.enter_context(tc.tile_pool(name="xload", bufs=3))
        for ch in range(NCH):
            s_acc = stats_p.tile([128, 2], fp32)
            nc.vector.memset(s_acc, 0.0)
            for t in range(H // 128):
                xt = xload_p.tile([128, W], fp32)
                nc.sync.dma_start(out=xt, in_=x_d[ch, t * 128:(t + 1) * 128, :])
                red = stats_p.tile([128, 2], fp32)
                nc.vector.tensor_reduce(red[:, 0:1], xt, axis=AX.X, op=OP.add)
                sq = xload_p.tile([128, W], fp32)
                nc.scalar.square(sq, xt)
                nc.vector.tensor_reduce(red[:, 1:2], sq, axis=AX.X, op=OP.add)
                nc.vector.tensor_tensor(s_acc, s_acc, red, OP.add)
            ps = psum_p.tile([1, 2], fp32)
            nc.tensor.matmul(out=ps, lhsT=ones_col, rhs=s_acc, start=True, stop=True)
            tot = stats_p.tile([1, 2], fp32)
            nc.vector.tensor_copy(out=tot, in_=ps)
            nel = float(H * W)
            m = stats_p.tile([1, 1], fp32)
            nc.scalar.mul(m, tot[:, 0:1], 1.0 / nel)
            t1 = stats_p.tile([1, 1], fp32)
            nc.vector.tensor_tensor(t1, tot[:, 0:1], m, OP.mult)
            t2 = stats_p.tile([1, 1], fp32)
            nc.vector.tensor_tensor(t2, tot[:, 1:2], t1, OP.subtract)
            var = stats_p.tile([1, 1], fp32)
            nc.scalar.mul(var, t2, 1.0 / (nel - 1.0))
            std = stats_p.tile([1, 1], fp32)
            nc.scalar.sqrt(std, var)
            sp = stats_p.tile([1, 1], fp32)
            nc.vector.tensor_scalar(out=sp, in0=std, scalar1=EPS, scalar2=None, op0=OP.add)
            s1 = stats_p.tile([1, 1], fp32)
            nc.vector.reciprocal(out=s1, in_=sp)
            nb = stats_p.tile([1, 1], fp32)
            nc.vector.tensor_tensor(nb, m, s1, OP.mult)
            nc.vector.tensor_copy(out=scal[:, ch * 4 + 0:ch * 4 + 1], in_=s1)
            nc.vector.tensor_scalar(out=scal[:, ch * 4 + 1:ch * 4 + 2], in0=nb,
                                    scalar1=-1.0, scalar2=None, op0=OP.mult)
            nc.vector.tensor_copy(out=scal[:, ch * 4 + 2:ch * 4 + 3], in_=std)
            nc.vector.tensor_copy(out=scal[:, ch * 4 + 3:ch * 4 + 4], in_=m)

        nc.gpsimd.partition_broadcast(bcast, scal)
        p1.close()

        def anticlamp_shift(dst, src, shift):
            # dst = relu(1 - relu(src + shift));  clamp01(t) = 1 - dst
            j = 9 - int(shift)
            nc.scalar.activation(out=dst, in_=src, func=AF.Relu,
                                 bias=shift_bias[:, j:j + 1], scale=1.0)
            nc.scalar.activation(out=dst, in_=dst, func=AF.Relu,
                                 bias=ones_col, scale=-1.0)

        # ---- pass 2: banded trapezoid convolution ----
        psum2_p = ctx.enter_context(tc.tile_pool(name="ps2", bufs=2, space="PSUM"))
        prod_p = ctx.enter_context(tc.tile_pool(name="prod", bufs=2))
        for _ in range(2):
            sha = sh_p.tile([128, NCH, WP], bf16, name="sha")
            nc.vector.memset(sha, 0.0)
            shb = sh_p.tile([128, NCH, WP], bf16, name="shb")
            nc.vector.memset(shb, 0.0)
        # edge bands skip the clamped-side halo (zero-weight taps), so they
        # emit 120 rows; 120 + 7*112 + 120 = 1024 in 9 bands instead of 10.
        bands = [(0, 120, 0, 0)]
        bands += [(120 + 112 * i, 112, 112 + 112 * i, 8) for i in range(7)]
        bands += [(904, 120, 896, 8)]
        for b, (r0, nrows, w0, p0) in enumerate(bands):
            v0 = max(0, -w0)
            v1 = min(128, H - w0)

            # kernel_sizes for output rows -> partitions 8..8+nrows
            kst = coords_p.tile([128, W, 2], fp32)
            if b == 0 or b == len(bands) - 1:
                nc.vector.memset(kst, 1.0)
            nc.sync.dma_start(out=kst[p0:p0 + nrows], in_=ks_d[r0:r0 + nrows, :, :])
            ksx = coords_p.tile([128, W], fp32)
            ksy = coords_p.tile([128, W], fp32)
            nc.vector.tensor_copy(out=ksx, in_=kst[:, :, 0])
            nc.vector.tensor_copy(out=ksy, in_=kst[:, :, 1])

            hcol = coords_p.tile([128, 1], fp32)
            nc.vector.tensor_scalar(out=hcol, in0=iota_col, scalar1=float(w0),
                                    scalar2=None, op0=OP.add)

            # window ends relative to the pixel (x axis: pos = wrow tensor)
            bxr = coords_p.tile([128, W], fp32)
            axr = coords_p.tile([128, W], fp32)
            tx = tmp_p.tile([128, W], fp32, bufs=1)
            nc.scalar.mul(tx, ksx, SC)                               # ksx*SC
            nc.vector.tensor_tensor(bxr, tx, wrow, OP.add)
            nc.vector.tensor_scalar(out=bxr, in0=bxr, scalar1=-SC, scalar2=None, op0=OP.add)
            nc.vector.tensor_scalar(out=bxr, in0=bxr, scalar1=0.0,
                                    scalar2=float(W - 1), op0=OP.max, op1=OP.min)
            nc.vector.tensor_tensor(bxr, bxr, wrow, OP.subtract)
            nc.scalar.mul(tx, ksx, -SC)
            nc.vector.tensor_tensor(axr, tx, wrow, OP.add)
            nc.vector.tensor_scalar(out=axr, in0=axr, scalar1=-SC, scalar2=None, op0=OP.add)
            nc.vector.tensor_scalar(out=axr, in0=axr, scalar1=0.0,
                                    scalar2=float(W - 1), op0=OP.max, op1=OP.min)
            nc.vector.tensor_tensor(axr, axr, wrow, OP.subtract)
            # y axis: pos = hcol per-partition scalar
            byr = coords_p.tile([128, W], fp32)
            ayr = coords_p.tile([128, W], fp32)
            nc.scalar.mul(tx, ksy, SC)
            nc.vector.tensor_scalar(out=byr, in0=tx, scalar1=hcol, scalar2=-SC,
                                    op0=OP.add, op1=OP.add)
            nc.vector.tensor_scalar(out=byr, in0=byr, scalar1=0.0,
                                    scalar2=float(H - 1), op0=OP.max, op1=OP.min)
            nc.vector.tensor_scalar(out=byr, in0=byr, scalar1=hcol, scalar2=None,
                                    op0=OP.subtract)
            nc.scalar.mul(tx, ksy, -SC)
            nc.vector.tensor_scalar(out=ayr, in0=tx, scalar1=hcol, scalar2=-SC,
                                    op0=OP.add, op1=OP.add)
            nc.vector.tensor_scalar(out=ayr, in0=ayr, scalar1=0.0,
                                    scalar2=float(H - 1), op0=OP.max, op1=OP.min)
            nc.vector.tensor_scalar(out=ayr, in0=ayr, scalar1=hcol, scalar2=None,
                                    op0=OP.subtract)

            # 1/(area+eps)
            rar = coords_p.tile([128, W], fp32)
            nc.vector.tensor_tensor(rar, ksx, ksy, OP.mult)
            nc.vector.tensor_scalar(out=rar, in0=rar, scalar1=EPS, scalar2=None, op0=OP.add)
            nc.vector.reciprocal(out=rar, in_=rar)

            # Wx planes (17), duplicated for both channels, bf16 storage
            wx = wx_p.tile([128, 17, W], bf16)
            qb = tmp_p.tile([128, W], bf16, bufs=1)
            qa = tmp_p.tile([128, W], bf16, bufs=1)
            for di, dv in enumerate(range(-8, 9)):
                anticlamp_shift(qb, bxr, 1 - dv)
                anticlamp_shift(qa, axr, 1 - dv)
                nc.vector.tensor_tensor(wx[:, di, :], qa, qb, OP.subtract)

            # normalized working tile, both channels: [128, NCH, WP]
            workf = work_p.tile([128, NCH, WP], fp32)
            nc.vector.memset(workf, 0.0)
            work = work_p.tile([128, NCH, WP], bf16)
            for ch in range(NCH):
                nc.sync.dma_start(out=workf[v0:v1, ch, PADW:PADW + W],
                                  in_=x_d[ch, w0 + v0:w0 + v1, :])
                nc.scalar.activation(out=work[:, ch, :], in_=workf[:, ch, :],
                                     func=AF.Identity,
                                     bias=bcast[:, ch * 4 + 1:ch * 4 + 2],
                                     scale=bcast[:, ch * 4 + 0:ch * 4 + 1])

            acc = acc_p.tile([128, NCH, W], bf16)
            nc.vector.memset(acc, 0.0)

            for r in range(-8, 9):
                a = max(0, -r)
                bb = 128 - max(0, r)
                if r == 0:
                    sha = work
                else:
                    sha = sh_p.tile([128, NCH, WP], bf16)
                    nc.sync.dma_start(out=sha[a:bb], in_=work[a + r:bb + r])
                shb = sh_p.tile([128, NCH, WP], bf16, name="shb")
                nc.sync.dma_start(out=shb[a:bb, :, 0:WP - 1],
                                  in_=work[a + r:bb + r, :, 1:WP])
                wy = tmp_p.tile([128, W], bf16)
                anticlamp_shift(qb, byr, 1 - r)
                anticlamp_shift(qa, ayr, 1 - r)
                nc.vector.tensor_tensor(wy, qa, qb, OP.subtract)

                pst = psum2_p.tile([128, NCH, W], fp32)
                # same-parity dv pairs fused into one 4D-AP product each
                groups = [(-8, -6), (-4, -2), (0, 2), (4, 6), (-7, -5),
                          (-3, -1), (1, 3), (5, 7), (8, None)]
                nmm = 0
                for d0, d1 in groups:
                    npair = 1 if d1 is None else 2
                    if (PADW + d0) % 2 == 0:
                        sht, e0 = sha, PADW + d0
                    else:
                        sht, e0 = shb, PADW + d0 - 1
                    srcv = bass.AP(tensor=sht.tensor, offset=sht.offset + e0,
                                   ap=[sht.ap[0], [2, npair], [WP, NCH], [1, W]])
                    wxi = wx[:, d0 + 8, :]
                    wxb = bass.AP(tensor=wxi.tensor, offset=wxi.offset,
                                  ap=[wxi.ap[0], [2 * W, npair], [0, NCH], [1, W]])
                    prod = prod_p.tile([128, 2, NCH, W], bf16)
                    pv = prod if npair == 2 else prod[:, 0:1]
                    nc.vector.tensor_tensor(pv, wxb, srcv, OP.mult)
                    for q in range(npair):
                        for hw in range(4):
                            nmm += 1
                            nc.tensor.matmul(
                                out=pst.rearrange("p a b -> p (a b)")[:, hw * 512:(hw + 1) * 512],
                                lhsT=ident,
                                rhs=prod[:, q].rearrange("p a b -> p (a b)")[:, hw * 512:(hw + 1) * 512],
                                start=(nmm <= 4), stop=(nmm > 64))
                pact = acc_p.tile([128, NCH, W], bf16)
                nc.scalar.activation(out=pact, in_=pst, func=AF.Copy, scale=1.0)
                wyb = bass.AP(tensor=wy.tensor, offset=wy.offset,
                              ap=[wy.ap[0], [0, NCH], [1, W]])
                t3m = acc_p.tile([128, NCH, W], bf16)
                nc.vector.tensor_tensor(t3m, wyb, pact, OP.mult)
                nc.vector.tensor_tensor(acc, acc, t3m, OP.add)

            for ch in range(NCH):
                outf = acc_p.tile([128, W], fp32)
                nc.vector.tensor_tensor(outf, acc[:, ch], rar, OP.mult)
                nc.scalar.activation(out=outf, in_=outf, func=AF.Identity,
                                     bias=bcast[:, ch * 4 + 3:ch * 4 + 4],
                                     scale=bcast[:, ch * 4 + 2:ch * 4 + 3])
                nc.sync.dma_start(out=out_d[ch, r0:r0 + nrows, :],
                                  in_=outf[p0:p0 + nrows])

    nc.compile()
    return nc


LAST_EXEC_NS = None
LAST_PROFILE = None


def kernel(x: np.ndarray, kernel_sizes: np.ndarray, _trace: bool = False) -> np.ndarray:
    global _COMPILED, LAST_EXEC_NS, LAST_PROFILE
    from concourse import bass_utils

    if _COMPILED is None:
        _COMPILED = build_bass()
    nc = _COMPILED

    x = np.ascontiguousarray(x, dtype=np.float32)
    ks = np.ascontiguousarray(kernel_sizes, dtype=np.float32)
    in_maps = []
    for core in range(8):
        n = core // 4
        c0 = (core % 4) * NCH
        in_maps.append({
            "x": np.ascontiguousarray(x[n, c0:c0 + NCH]),
            "kernel_sizes": ks[n],
        })
    res = bass_utils.run_bass_kernel_spmd(nc, in_maps, core_ids=list(range(8)),
                                          trace=_trace)
    LAST_EXEC_NS = res.exec_time_ns
    LAST_PROFILE = res.profile_json
    out = np.empty((N, C, H, W), dtype=np.float32)
    for core in range(8):
        n = core // 4
        c0 = (core % 4) * NCH
        out[n, c0:c0 + NCH] = res.results[core]["out"].reshape(NCH, H, W)
    return out
